# revision 5
# baseline (speedup 1.0000x reference)
"""Trainium2 Bass kernel for nn_ContrastiveRegressiveModel_88665304859078.

Reference math (M=4, P=128, C=128, N=2P=256):
    lab   = labels.transpose(1,0,2).reshape(M, N)
    batch = concat([q, k], axis=1)                       # [M, N, C]
    dists[m,i,j] = |lab[m,i] - lab[m,j]|
    sims[m,i,j]  = exp(-L1(batch[m,i], batch[m,j])) off-diag, 0 on diag
    pt[m,i,j,l]  = (dists[m,i,j] >= dists[m,i,l]) & (i != l)
    sum3[m,i,l]  = sum_j sims[m,i,j]*pt[m,i,j,l] + I[i,l]
    interim      = log(sims/sum3 + I)
    loss = (sum(interim[:,0]) + sum(interim[:,P])) * (-1/(M*(P-1))) / 2

Only rows i=0 and i=P of `interim` reach the loss, so the N^3 einsum
collapses to two row problems per batch element: 8 (m, i) pairs total,
one per NeuronCore.  Each core receives (host-side data marshaling
only — slices/transpose/concat of the raw inputs):
    a    [128,128]  the half of batch holding the center row (row 0)
    b    [128,128]  the other half
    labs [128,2]    labels in column layout (col 0 = a-side, col 1 = b-side)
    labr [1,256]    labels in row layout (a-side then b-side)
and computes  partial = sum_l log(s_l / sum3_l)  for its row, where
    d_x   = |lab_x - lab_0|
    s_x   = exp(-L1(batch_x, batch_0)), s_0 = 0
    sum3_l = sum_j s_j * (d_j >= d_l)
The i=P cores simply get (a,b) = (k[m], q[m]) — the row sum is
invariant under that relabeling.  Host combines the 8 scalars.
"""

import sys

for _p in ("/root/.axon_site/_ro/trn_rl_repo", "/opt/trn_rl_repo"):
    if _p not in sys.path:
        sys.path.append(_p)

from contextlib import ExitStack

import numpy as np

import concourse.bass as bass
import concourse.tile as tile
from concourse import bacc, mybir
from concourse._compat import with_exitstack
from concourse.bass_utils import run_bass_kernel_spmd
from concourse.masks import make_identity

M, P, C = 4, 128, 128
N = 2 * P
F32 = mybir.dt.float32

# Results of the last run_bass_kernel_spmd call (for test harnesses to
# read exec_time_ns after calling kernel()).
LAST_RESULT = None


@with_exitstack
def _row_loss_kernel(
    ctx: ExitStack,
    tc: tile.TileContext,
    out_ap: bass.AP,
    a_ap: bass.AP,
    b_ap: bass.AP,
    labs_ap: bass.AP,
    labr_ap: bass.AP,
):
    nc = tc.nc
    sb = ctx.enter_context(tc.tile_pool(name="sb", bufs=1))
    ps = ctx.enter_context(tc.tile_pool(name="ps", bufs=1, space="PSUM"))

    # ---- loads -----------------------------------------------------------
    AB = sb.tile([P, 2, C], F32)  # [:,0,:] = a rows, [:,1,:] = b rows
    nc.sync.dma_start(out=AB[:, 0, :], in_=a_ap)
    nc.sync.dma_start(out=AB[:, 1, :], in_=b_ap)
    Lc = sb.tile([P, 2], F32)
    nc.sync.dma_start(out=Lc, in_=labs_ap)
    Lr = sb.tile([1, N], F32)
    nc.sync.dma_start(out=Lr, in_=labr_ap)

    ones_r = sb.tile([1, P], F32)
    nc.vector.memset(ones_r, 1.0)
    ident = sb.tile([P, P], F32)
    make_identity(nc, ident)

    # ---- broadcast center vector / center label over partitions ----------
    V = ps.tile([P, C], F32)  # V[p, c] = a[0, c] for every p
    nc.tensor.matmul(V, lhsT=ones_r, rhs=AB[0:1, 0, :], start=True, stop=True)
    Cb = ps.tile([P, 1], F32)  # Cb[p] = lab_0
    nc.tensor.matmul(Cb, lhsT=ones_r, rhs=Lr[0:1, 0:1], start=True, stop=True)
    Cbs = sb.tile([P, 1], F32)
    nc.vector.tensor_copy(out=Cbs, in_=Cb)

    # ---- label distances in both layouts ---------------------------------
    # Dr[0, x] = |lab_x - lab_0|
    Dr = sb.tile([1, N], F32)
    nc.vector.tensor_scalar(
        out=Dr, in0=Lr, scalar1=Lr[0:1, 0:1], scalar2=None,
        op0=mybir.AluOpType.subtract,
    )
    nc.scalar.activation(out=Dr, in_=Dr, func=mybir.ActivationFunctionType.Abs)
    # Dc[p, h] = |lab_{p + 128h} - lab_0|  (column layout)
    Dc = sb.tile([P, 2], F32)
    nc.vector.tensor_scalar(
        out=Dc, in0=Lc, scalar1=Cbs, scalar2=None,
        op0=mybir.AluOpType.subtract,
    )
    nc.scalar.activation(out=Dc, in_=Dc, func=mybir.ActivationFunctionType.Abs)
    # DrB[p, l] = d_l for every partition p
    DrB = ps.tile([P, N], F32)
    nc.tensor.matmul(DrB, lhsT=ones_r, rhs=Dr, start=True, stop=True)

    # ---- threshold masks: Mh[j, l] = (d_j >= d_l), j in half h -----------
    Mq = sb.tile([P, N], F32)
    nc.vector.tensor_scalar(
        out=Mq, in0=DrB, scalar1=Dc[:, 0:1], scalar2=None,
        op0=mybir.AluOpType.is_le,
    )
    Mk = sb.tile([P, N], F32)
    nc.vector.tensor_scalar(
        out=Mk, in0=DrB, scalar1=Dc[:, 1:2], scalar2=None,
        op0=mybir.AluOpType.is_le,
    )

    # ---- similarities s_j = exp(-sum_c |batch_j - batch_0|) --------------
    T = sb.tile([P, 2, C], F32)
    nc.vector.tensor_tensor(
        out=T[:, 0, :], in0=AB[:, 0, :], in1=V, op=mybir.AluOpType.subtract
    )
    nc.vector.tensor_tensor(
        out=T[:, 1, :], in0=AB[:, 1, :], in1=V, op=mybir.AluOpType.subtract
    )
    L1 = sb.tile([P, 2], F32)
    nc.vector.tensor_reduce(
        out=L1, in_=T, axis=mybir.AxisListType.X, op=mybir.AluOpType.add,
        apply_absolute_value=True,
    )
    S = sb.tile([P, 2], F32)
    nc.scalar.activation(out=S, in_=L1, func=mybir.ActivationFunctionType.Exp,
                         scale=-1.0)
    # self-similarity (j = 0) is zero in the reference
    nc.vector.memset(S[0:1, 0:1], 0.0)

    # ---- sum3[l] = sum_j s_j * mask[j, l]  (row layout, PSUM accum) ------
    S3 = ps.tile([1, N], F32)
    nc.tensor.matmul(S3, lhsT=S[:, 0:1], rhs=Mq, start=True, stop=False)
    nc.tensor.matmul(S3, lhsT=S[:, 1:2], rhs=Mk, start=False, stop=True)

    # ---- s transposed to row layout via identity matmuls -----------------
    Sr = ps.tile([1, N], F32)
    nc.tensor.matmul(Sr[0:1, 0:P], lhsT=S[:, 0:1], rhs=ident, start=True, stop=True)
    nc.tensor.matmul(Sr[0:1, P:N], lhsT=S[:, 1:2], rhs=ident, start=True, stop=True)

    # ---- interim row: log(s_l / sum3_l), l=0 term is exactly 0 ----------
    # IEEE semantics of the reference:  s=0,sum3=0 -> NaN;  s=0,sum3>0 ->
    # -inf;  else finite.  Feeding NaN/inf through Ln is undefined on the
    # scalar engine, so compute a strictly-positive "safe" log and inject
    # the -inf/NaN terms with copy_predicated.
    FLT_MIN = 1.17549435e-38  # smallest normal f32
    m_s0 = sb.tile([1, N], mybir.dt.uint8)  # 1 where s_l == 0
    nc.vector.tensor_scalar(
        out=m_s0, in0=Sr, scalar1=0.0, scalar2=None, op0=mybir.AluOpType.is_equal
    )
    m_30 = sb.tile([1, N], mybir.dt.uint8)  # 1 where sum3_l == 0
    nc.vector.tensor_scalar(
        out=m_30, in0=S3, scalar1=0.0, scalar2=None, op0=mybir.AluOpType.is_equal
    )
    # clamp into normal range; clamped entries are exactly the masked ones,
    # which copy_predicated overwrites below
    safe_s = sb.tile([1, N], F32)
    nc.vector.tensor_scalar(
        out=safe_s, in0=Sr, scalar1=FLT_MIN, scalar2=None, op0=mybir.AluOpType.max
    )
    safe_3 = sb.tile([1, N], F32)
    nc.vector.tensor_scalar(
        out=safe_3, in0=S3, scalar1=FLT_MIN, scalar2=None, op0=mybir.AluOpType.max
    )
    R3 = sb.tile([1, N], F32)
    nc.vector.reciprocal(out=R3, in_=safe_3)
    Rat = sb.tile([1, N], F32)
    nc.vector.tensor_tensor(out=Rat, in0=safe_s, in1=R3, op=mybir.AluOpType.mult)
    Lg = sb.tile([1, N], F32)
    nc.scalar.activation(out=Lg, in_=Rat, func=mybir.ActivationFunctionType.Ln)
    ninf = sb.tile([1, N], F32)
    nc.vector.memset(ninf, float("-inf"))
    nan_t = sb.tile([1, N], F32)
    nc.vector.memset(nan_t, float("nan"))
    nc.vector.copy_predicated(out=Lg, mask=m_s0, data=ninf)
    nc.vector.copy_predicated(out=Lg, mask=m_30, data=nan_t)
    nc.vector.memset(Lg[0:1, 0:1], 0.0)

    Fin = sb.tile([1, 1], F32)
    nc.vector.tensor_reduce(
        out=Fin, in_=Lg, axis=mybir.AxisListType.X, op=mybir.AluOpType.add
    )
    nc.sync.dma_start(out=out_ap, in_=Fin)


def build_program():
    nc = bacc.Bacc("TRN2", target_bir_lowering=False, debug=False, num_devices=8)
    a = nc.dram_tensor("a", [P, C], F32, kind="ExternalInput").ap()
    b = nc.dram_tensor("b", [P, C], F32, kind="ExternalInput").ap()
    labs = nc.dram_tensor("labs", [P, 2], F32, kind="ExternalInput").ap()
    labr = nc.dram_tensor("labr", [1, N], F32, kind="ExternalInput").ap()
    out = nc.dram_tensor("out", [1, 1], F32, kind="ExternalOutput").ap()
    with tile.TileContext(nc) as tc:
        _row_loss_kernel(tc, out, a, b, labs, labr)
    nc.compile()
    return nc


_NC_CACHE = None


def _get_program():
    global _NC_CACHE
    if _NC_CACHE is None:
        _NC_CACHE = build_program()
    return _NC_CACHE


def _make_in_maps(q, k, labels):
    q = np.asarray(q, dtype=np.float32)
    k = np.asarray(k, dtype=np.float32)
    labels = np.asarray(labels, dtype=np.float32)
    in_maps = []
    for m in range(M):
        for side in range(2):  # 0 -> center is q[m,0] (i=0); 1 -> k[m,0] (i=P)
            if side == 0:
                a, b = q[m], k[m]
                la, lb = labels[0, m], labels[1, m]
            else:
                a, b = k[m], q[m]
                la, lb = labels[1, m], labels[0, m]
            in_maps.append({
                "a": np.ascontiguousarray(a),
                "b": np.ascontiguousarray(b),
                "labs": np.ascontiguousarray(np.stack([la, lb], axis=1)),
                "labr": np.concatenate([la, lb]).reshape(1, N),
            })
    return in_maps


def kernel(q, k, labels):
    global LAST_RESULT
    nc = _get_program()
    in_maps = _make_in_maps(q, k, labels)
    res = run_bass_kernel_spmd(nc, in_maps, core_ids=list(range(8)))
    LAST_RESULT = res
    partials = np.array(
        [res.results[c]["out"][0, 0] for c in range(8)], dtype=np.float32
    )
    scale = np.float32(-1.0 / (M * (P - 1)))
    sum_q = partials[0::2].sum(dtype=np.float32)  # rows i=0
    sum_k = partials[1::2].sum(dtype=np.float32)  # rows i=P
    return np.float32((sum_q * scale + sum_k * scale) / np.float32(2.0))


# revision 6
# speedup vs baseline: 1.2479x; 1.2479x over previous
"""Optimized v2 of the row-loss kernel (see kernel.py docstring for math).

Changes vs v1:
- inputs packed host-side into ab=[128,256] (one big DMA) + labr=[1,256]
- center vector/label broadcast via 0-partition-stride DMAs (frees PE,
  overlaps the main load) instead of ones-matmuls
- label column layout derived on-chip with two K=1 transpose matmuls
  (drops the labs input)
- mask / s matmuls in bf16 (masks are exact 0/1; s rounding ~0.4% << tol)
- single fused |AB - V| subtract via a 0-stride repeat of V
- slimmer tail: Ln(0) supplies the -inf branch naturally; only the
  sum3==0 -> NaN patch needs copy_predicated
"""
import sys

for _p in ("/root/.axon_site/_ro/trn_rl_repo", "/opt/trn_rl_repo"):
    if _p not in sys.path:
        sys.path.append(_p)

from contextlib import ExitStack

import numpy as np

import concourse.bass as bass
import concourse.tile as tile
from concourse import bacc, mybir
from concourse._compat import with_exitstack
from concourse.bass_utils import run_bass_kernel_spmd
from concourse.masks import make_identity

M, P, C = 4, 128, 128
N = 2 * P
F32 = mybir.dt.float32
BF16 = mybir.dt.bfloat16

LAST_RESULT = None


def _rep2(ap):
    """[p, n] AP -> [p, 2, n] AP with a 0-stride middle dim."""
    return bass.AP(tensor=ap.tensor, offset=ap.offset,
                   ap=[list(ap.ap[0]), [0, 2], list(ap.ap[1])])


@with_exitstack
def _row_loss_kernel(ctx: ExitStack, tc: tile.TileContext, out_ap, ab_ap, labr_ap):
    nc = tc.nc
    sb = ctx.enter_context(tc.tile_pool(name="sb", bufs=1))
    ps = ctx.enter_context(tc.tile_pool(name="ps", bufs=1, space="PSUM"))

    # ---- loads (labr first: its completion gates the critical label
    # chain; the big AB load overlaps on the same HWDGE engine) -----------
    Lr = sb.tile([1, N], F32)
    nc.sync.dma_start(out=Lr, in_=labr_ap)
    AB = sb.tile([P, 2, C], F32)
    nc.scalar.dma_start(out=AB, in_=ab_ap.rearrange("p (h c) -> p h c", h=2))
    V = sb.tile([P, C], F32)  # center vector ab[0, 0:128] on every partition
    nc.gpsimd.dma_start(
        out=V,
        in_=bass.AP(tensor=ab_ap.tensor, offset=ab_ap.offset, ap=[[0, P], [1, C]]),
    )

    # ---- input-independent constants (scheduled into DMA wait time) ------
    ones_r = sb.tile([1, P], BF16)
    nc.vector.memset(ones_r, 1.0)
    one11 = sb.tile([1, 1], BF16)
    nc.vector.memset(one11, 1.0)
    ident = sb.tile([P, P], BF16)
    make_identity(nc, ident)

    # ---- label distances (bf16 values; comparisons are consistent because
    # both DrB and Dc hold exact f32 copies of the same bf16 numbers) ------
    Dr = sb.tile([1, N], F32)
    nc.vector.tensor_scalar(out=Dr, in0=Lr, scalar1=Lr[0:1, 0:1], scalar2=None,
                            op0=mybir.AluOpType.subtract)
    Dr_bf = sb.tile([1, N], BF16)  # |lab_x - lab_0| rounded to bf16
    nc.scalar.activation(out=Dr_bf, in_=Dr, func=mybir.ActivationFunctionType.Abs)
    DrB = ps.tile([P, N], F32)  # DrB[p, l] = d_l on every partition
    nc.tensor.matmul(DrB, lhsT=ones_r, rhs=Dr_bf, start=True, stop=True)
    Dc_ps = ps.tile([P, 2], F32)  # column layout via K=1 transpose matmuls
    nc.tensor.matmul(Dc_ps[:, 0:1], lhsT=Dr_bf[0:1, 0:P], rhs=one11, start=True, stop=True)
    nc.tensor.matmul(Dc_ps[:, 1:2], lhsT=Dr_bf[0:1, P:N], rhs=one11, start=True, stop=True)

    # ---- threshold masks Mh[j, l] = (d_l <= d_j), bf16 0/1 ---------------
    Mq = sb.tile([P, N], BF16)
    nc.vector.tensor_scalar(out=Mq, in0=DrB, scalar1=Dc_ps[:, 0:1], scalar2=None,
                            op0=mybir.AluOpType.is_le)
    Mk = sb.tile([P, N], BF16)
    nc.vector.tensor_scalar(out=Mk, in0=DrB, scalar1=Dc_ps[:, 1:2], scalar2=None,
                            op0=mybir.AluOpType.is_le)

    # ---- similarities s_j = exp(-sum_c |batch_j - batch_0|) --------------
    T = sb.tile([P, 2, C], F32)
    nc.vector.tensor_tensor(out=T, in0=AB, in1=_rep2(V[:]), op=mybir.AluOpType.subtract)
    L1 = sb.tile([P, 2], F32)
    nc.vector.tensor_reduce(out=L1, in_=T, axis=mybir.AxisListType.X,
                            op=mybir.AluOpType.add, apply_absolute_value=True)
    S = sb.tile([P, 2], BF16)
    nc.scalar.activation(out=S, in_=L1, func=mybir.ActivationFunctionType.Exp,
                         scale=-1.0)
    nc.vector.memset(S[0:1, 0:1], 0.0)  # self-similarity is zero

    # ---- sum3 row + s row (PSUM) -----------------------------------------
    S3 = ps.tile([1, N], F32)
    nc.tensor.matmul(S3, lhsT=S[:, 0:1], rhs=Mq, start=True, stop=False)
    nc.tensor.matmul(S3, lhsT=S[:, 1:2], rhs=Mk, start=False, stop=True)
    Sr = ps.tile([1, N], F32)
    nc.tensor.matmul(Sr[0:1, 0:P], lhsT=S[:, 0:1], rhs=ident, start=True, stop=True)
    nc.tensor.matmul(Sr[0:1, P:N], lhsT=S[:, 1:2], rhs=ident, start=True, stop=True)

    # ---- interim row: log(s_l / sum3_l) ----------------------------------
    # R3 = 1/sum3: inf where sum3==0.  Ln sees the CLAMPED ratio (finite,
    # and 0 where s==0 -> Ln gives the reference's -inf).  The NaN branch
    # (sum3==0, i.e. 0/0) is injected afterwards as P = R3 - R3, which is
    # NaN exactly where R3 is inf and 0.0 elsewhere.
    R3 = sb.tile([1, N], F32)
    nc.vector.reciprocal(out=R3, in_=S3)
    Rat = sb.tile([1, N], F32)
    nc.vector.scalar_tensor_tensor(out=Rat, in0=R3, scalar=3.0e38, in1=Sr,
                                   op0=mybir.AluOpType.min,
                                   op1=mybir.AluOpType.mult)
    # l=0 patched BEFORE Ln: Ln(1)=0, so the fused accumulation below needs
    # no post-hoc memset.  (If sum3[0]=0 then ALL s are 0 and every column
    # is NaN anyway, so Pn[0]'s NaN is consistent with the reference.)
    nc.vector.memset(Rat[0:1, 0:1], 1.0)
    Lg = sb.tile([1, N], F32)
    nc.scalar.activation(out=Lg, in_=Rat, func=mybir.ActivationFunctionType.Ln)
    Pn = sb.tile([1, N], F32)
    nc.vector.tensor_tensor(out=Pn, in0=R3, in1=R3, op=mybir.AluOpType.subtract)
    nc.vector.memset(Pn[0:1, 0:1], 0.0)  # l=0 contributes exactly 0
    LgF = sb.tile([1, N], F32)
    Fin = sb.tile([1, 1], F32)
    nc.vector.scalar_tensor_tensor(out=LgF, in0=Lg, scalar=0.0, in1=Pn,
                                   op0=mybir.AluOpType.add,
                                   op1=mybir.AluOpType.add, accum_out=Fin)
    nc.sync.dma_start(out=out_ap, in_=Fin)


def build_program():
    nc = bacc.Bacc("TRN2", target_bir_lowering=False, debug=False, num_devices=8)
    ab = nc.dram_tensor("ab", [P, N], F32, kind="ExternalInput").ap()
    labr = nc.dram_tensor("labr", [1, N], F32, kind="ExternalInput").ap()
    out = nc.dram_tensor("out", [1, 1], F32, kind="ExternalOutput").ap()
    with tile.TileContext(nc) as tc:
        _row_loss_kernel(tc, out, ab, labr)
    nc.compile()
    return nc


_NC_CACHE = None


def _get_program():
    global _NC_CACHE
    if _NC_CACHE is None:
        _NC_CACHE = build_program()
    return _NC_CACHE


def _make_in_maps(q, k, labels):
    q = np.asarray(q, dtype=np.float32)
    k = np.asarray(k, dtype=np.float32)
    labels = np.asarray(labels, dtype=np.float32)
    in_maps = []
    for m in range(M):
        for side in range(2):  # 0 -> center q[m,0] (row i=0); 1 -> k[m,0] (i=P)
            if side == 0:
                a, b = q[m], k[m]
                la, lb = labels[0, m], labels[1, m]
            else:
                a, b = k[m], q[m]
                la, lb = labels[1, m], labels[0, m]
            in_maps.append({
                "ab": np.ascontiguousarray(np.concatenate([a, b], axis=1)),
                "labr": np.concatenate([la, lb]).reshape(1, N),
            })
    return in_maps


def kernel(q, k, labels):
    global LAST_RESULT
    nc = _get_program()
    in_maps = _make_in_maps(q, k, labels)
    res = run_bass_kernel_spmd(nc, in_maps, core_ids=list(range(8)))
    LAST_RESULT = res
    partials = np.array(
        [res.results[c]["out"][0, 0] for c in range(8)], dtype=np.float32
    )
    scale = np.float32(-1.0 / (M * (P - 1)))
    sum_q = partials[0::2].sum(dtype=np.float32)
    sum_k = partials[1::2].sum(dtype=np.float32)
    return np.float32((sum_q * scale + sum_k * scale) / np.float32(2.0))
